# revision 36
# baseline (speedup 1.0000x reference)
"""Trainium2 Bass kernel for nn_AttentionBlock (ragged_sequence, 16 equal
segments of 2048 q/kv tokens, HID=256, QD=64) on 8 NeuronCores.

Sharding: 2 segments (4096 rows) per core, weights replicated, outputs
concatenated host-side (attention is block-diagonal per segment -> no
cross-core communication needed).

v2 design on top of the fp8/DoubleRow baseline:
 - K=64 score matmuls run 2x via PE row-tiling: even j-tiles use SBUF
   partitions 0-63 / tile_position (0,0), odd j-tiles a DMA-duplicated
   copy on partitions 64-127 / (64,0) - concurrent row-band tiles
   (measured 3.3x on the score shape in isolation).
 - One [128,2048] exp ACTIVATE per j-tile pair (amortizes the ~290ns
   PSUM access tax per instruction).
 - zT for the fc layer comes from dma_start_transpose (SBUF->SBUF
   xbar), not PE transposes; z/zT for a chunk are produced a full
   pipeline iteration before the fc stage consumes them (4-stage
   pipeline) so the DMA latency is hidden.
 - Per-jt [128,1024] st tiles (2 PSUM bufs) keep the score->exp->score
   WAR chain off the critical path; segment-1 projections are
   interleaved into pipeline iteration 0; q is streamed bf16.
"""

import os
import sys

os.environ.setdefault("MYCRO_LOCAL_CACHE", "1")
if "/opt/trn_rl_repo" not in sys.path:
    sys.path.insert(0, "/opt/trn_rl_repo")

import numpy as np

HID = 256
QD = 64
LQ = 2048
LH = 2048
B = 16
NCORES = 8
SEGS = 2                  # segments per core
ROWS = SEGS * LQ          # 4096 q rows per core
EPS = 1e-5
SCALE = 1.0 / 8.0         # 1/sqrt(QD)

_built = {}               # (apply0,) -> nc


def _patch_act_tables():
    """Make the act-table pass choose the combined exp+ln table for every
    activation: blank all other tables (indices preserved so walrus's
    act_func_set_id remap stays correct). Avoids 100+ ACT_TABLE_LOADs
    (1.28us each) from alternating Exp/Ln table picks."""
    import functools
    import concourse.hw_specs as hw_specs
    import concourse.bacc as bacc_mod
    if getattr(hw_specs, "_attn_tables_patched", False):
        return
    orig = hw_specs.get_activation_tables

    @functools.cache
    def patched(arch):
        tabs = dict(orig(arch))
        joint = "natural_log_exp_and_others"
        assert joint in tabs, sorted(tabs)
        return {name: (funcs if name == joint else set())
                for name, funcs in tabs.items()}

    hw_specs.get_activation_tables = patched
    bacc_mod.get_activation_tables = patched
    hw_specs._attn_tables_patched = True


def _build(apply0: bool):
    from concourse import bacc, bass, mybir, tile

    _patch_act_tables()

    dt = mybir.dt
    f32 = dt.float32
    bf16 = dt.bfloat16
    f8 = dt.float8e4
    AF = mybir.ActivationFunctionType
    Alu = mybir.AluOpType
    DR = mybir.MatmulPerfMode.DoubleRow

    nc = bacc.Bacc("TRN2", target_bir_lowering=False, debug=False,
                   enable_asserts=False)

    qT8_d = nc.dram_tensor("qT8", [HID, ROWS], f8, kind="ExternalInput")
    hT8_d = nc.dram_tensor("hT8", [HID, ROWS], f8, kind="ExternalInput")
    qbf_d = nc.dram_tensor("qbf", [ROWS, HID], bf16, kind="ExternalInput")
    wq8_d = nc.dram_tensor("WQ8", [128, 2 * QD], f8, kind="ExternalInput")
    wk8_d = nc.dram_tensor("WK8", [128, 2 * QD], f8, kind="ExternalInput")
    wv8_d = nc.dram_tensor("WV8", [128, 2 * HID], f8, kind="ExternalInput")
    fwT_d = nc.dram_tensor("FCWT", [HID, HID], bf16, kind="ExternalInput")
    fb_d = nc.dram_tensor("FCB", [1, HID], bf16, kind="ExternalInput")
    if apply0:
        n0w_d = nc.dram_tensor("N0W", [128, HID], f32, kind="ExternalInput")
        n0b_d = nc.dram_tensor("N0B", [128, HID], f32, kind="ExternalInput")
    out_d = nc.dram_tensor("out", [ROWS, HID], bf16, kind="ExternalOutput")

    qbf_a = qbf_d.ap()
    out_a = out_d.ap()

    NJT = LH // 128           # 16 j-tiles per segment
    NIC = 2                   # 1024-col i-chunks per segment for scores
    ICW = LQ // NIC           # 1024
    NIL = ICW // 128          # 8 i-tiles per chunk
    VW = HID + 1              # V block width incl ones column

    with tile.TileContext(nc) as tc:
        with (
            tc.tile_pool(name="const", bufs=1) as cpool,
            tc.tile_pool(name="kqq", bufs=1) as kqq_pool,
            tc.tile_pool(name="vsb", bufs=1) as v_pool,
        ):
            # ---- constants ----
            wq_sb = cpool.tile([128, 2 * QD], f8)
            wk_sb = cpool.tile([128, 2 * QD], f8)
            wv_sb = cpool.tile([128, 2 * HID], f8)
            fw_sb = cpool.tile([128, 2 * HID], bf16)    # fc_w.T chunks
            fb_sb = cpool.tile([1, HID], bf16)
            one_sb = cpool.tile([1, 128], bf16)
            def load_consts():
                nc.sync.dma_start(wk_sb[:], wk8_d.ap()[:, :])
                nc.sync.dma_start(wq_sb[:], wq8_d.ap()[:, :])
                nc.sync.dma_start(wv_sb[:], wv8_d.ap()[:, :])
                for e in range(2):
                    nc.sync.dma_start(fw_sb[:, e * HID:(e + 1) * HID],
                                      fwT_d.ap()[e * 128:(e + 1) * 128, :])
                nc.sync.dma_start(fb_sb[:], fb_d.ap()[:, :])
            nc.vector.memset(one_sb[:], 1.0)
            eps_sb = cpool.tile([128, 1], f32)
            nc.vector.memset(eps_sb[:], EPS)
            nb3_sb = cpool.tile([128, 1], f32)
            nc.vector.memset(nb3_sb[:], -3.0)
            cachebust = cpool.tile([1, 1], f32)
            nc.vector.memset(cachebust[:], 12.25)
            if apply0:
                n0w_sb = cpool.tile([128, HID], f32)
                n0b_sb = cpool.tile([128, HID], f32)
                nc.sync.dma_start(n0w_sb[:], n0w_d.ap()[:, :])
                nc.sync.dma_start(n0b_sb[:], n0b_d.ap()[:, :])

            # persistent activations. kqq layout: [128, 2*ROWS] bf16 with
            # cols [0,ROWS)=K^T, [ROWS,2*ROWS)=qq^T; partitions 0-63 are
            # the real data, 64-127 a DMA duplicate for the odd-jt PE band.
            kqq_sb = kqq_pool.tile([128, 2 * ROWS], bf16)
            v_sb = v_pool.tile([128, SEGS * NJT * VW], f8)

            # ---- main context: projections are emitted per-segment and
            # segment 1 is interleaved into pipeline iteration 0 so the PE
            # has projection work between the first score/exp pairs.
            with (
                tc.tile_pool(name="qhT", bufs=1) as qh_pool,
                tc.tile_pool(name="pt", bufs=2) as pt_pool,
                tc.tile_pool(name="qrow", bufs=6) as q_pool,
                tc.tile_pool(name="xs", bufs=3) as xs_pool,
                tc.tile_pool(name="ys", bufs=3) as ys_pool,
                tc.tile_pool(name="zt", bufs=3) as z_pool,
                tc.tile_pool(name="zT", bufs=3) as zT_pool,
                tc.tile_pool(name="st8", bufs=3) as st8_pool,
                tc.tile_pool(name="outp", bufs=6) as o_pool,
                tc.tile_pool(name="ps_st", bufs=2,
                             space=bass.MemorySpace.PSUM) as ps_st,
                tc.tile_pool(name="ps_att", bufs=2,
                             space=bass.MemorySpace.PSUM) as ps_att,
                tc.tile_pool(name="ps_fc", bufs=2,
                             space=bass.MemorySpace.PSUM) as ps_fc,
            ):
                q8t = qh_pool.tile([128, 2 * ROWS], f8, tag="q8")
                h8t = qh_pool.tile([128, 2 * ROWS], f8, tag="h8")
                # segment-major, h before q so K^T and V projections of
                # a segment never wait behind q transfers
                for seg in range(SEGS):
                    for tens, dram in ((h8t, hT8_d), (q8t, qT8_d)):
                        for c in range(seg * LQ, (seg + 1) * LQ, 1024):
                            for e in range(2):
                                nc.sync.dma_start(
                                    tens[:, e * ROWS + c:e * ROWS + c + 1024],
                                    dram.ap()[e * 128:(e + 1) * 128,
                                              c:c + 1024])
                        if seg == 0 and tens is h8t:
                            load_consts()
                q8r = q8t[:].rearrange("p (e c) -> p e c", e=2)
                h8r = h8t[:].rearrange("p (e c) -> p e c", e=2)
                wqr = wq_sb[:].rearrange("p (e m) -> p e m", e=2)
                wkr = wk_sb[:].rearrange("p (e m) -> p e m", e=2)
                wvr = wv_sb[:].rearrange("p (e m) -> p e m", e=2)

                # kT / qq: [64, 512] chunks; copy casts f32->bf16 into band
                # 0, then an SBUF->SBUF DMA duplicates into band 1
                # (partitions 64-127) for the odd-jt row-tile. PSUM comes
                # from the fc pool ([0:64, :] slice of a [128,512] tile).
                ncopy = [0]

                def kq_chunk(dstoff, w_r, src, col):
                    ps = ps_fc.tile([128, 512], f32, tag="fc")
                    nc.tensor.matmul(ps[0:64, :], w_r,
                                     src[:, :, col:col + 512],
                                     start=True, stop=True, perf_mode=DR)
                    dst = kqq_sb[0:64, dstoff + col:dstoff + col + 512]
                    if ncopy[0] % 2 == 0:
                        nc.scalar.copy(dst, ps[0:64, :])
                    else:
                        nc.vector.tensor_copy(dst, ps[0:64, :])
                    ncopy[0] += 1
                    nc.sync.dma_start(
                        kqq_sb[64:128, dstoff + col:dstoff + col + 512],
                        kqq_sb[0:64, dstoff + col:dstoff + col + 512])

                def v_pair(s, jt2):
                    ps = ps_att.tile([128, 512], f32, tag="att")
                    for u in range(2):
                        col = s * LH + (2 * jt2 + u) * 128
                        nc.tensor.matmul(ps[:, u * HID:(u + 1) * HID],
                                         h8r[:, :, col:col + 128],
                                         wvr, start=True, stop=True,
                                         perf_mode=DR)
                    base = (s * NJT + 2 * jt2) * VW
                    dst = v_sb[:, base:base + 2 * VW] \
                        .rearrange("p (two d) -> p two d", two=2)
                    nc.vector.tensor_copy(
                        dst[:, :, 0:HID],
                        ps[:, 0:2 * HID]
                        .rearrange("p (two d) -> p two d", two=2))
                    for u in range(2):
                        nc.vector.memset(
                            v_sb[:, base + u * VW + HID:
                                 base + (u + 1) * VW], 1.0)

                def emit_proj_piece(s, piece):
                    """pieces 0-7: kT/qq col chunks; 8-15: V jt pairs."""
                    if piece < 4:
                        kq_chunk(0, wkr, h8r, s * LQ + piece * 512)
                    elif piece < 8:
                        kq_chunk(ROWS, wqr, q8r, s * LQ + (piece - 4) * 512)
                    else:
                        v_pair(s, piece - 8)

                # segment 0 projections up front
                for piece in range(16):
                    emit_proj_piece(0, piece)

                def emit_score_pair(ep, jp):
                    """Two j-tiles on opposite PE row bands, each into its
                    own [128,1024] st tile with its own exp (double-buffered
                    so the next pair's matmuls never wait a full exp)."""
                    s, ic, pt = ep["s"], ep["ic"], ep["pt"]
                    icol = ROWS + s * LQ + ic * ICW
                    for u in range(2):
                        jt = 2 * jp + u
                        b = 64 * u
                        st = ps_st.tile([128, 1024], f32, tag="st")
                        for h in range(2):
                            nc.tensor.matmul(
                                st[:, h * 512:(h + 1) * 512],
                                kqq_sb[b:b + 64,
                                       s * LH + jt * 128:
                                       s * LH + (jt + 1) * 128],
                                kqq_sb[b:b + 64,
                                       icol + h * 512:icol + (h + 1) * 512],
                                start=True, stop=True,
                                tile_position=(b, 0))
                        nc.scalar.activation(
                            pt[:, jt * ICW:(jt + 1) * ICW],
                            st[:], AF.Exp, scale=SCALE, bias=nb3_sb[:])

                def emit_a(ep, il):
                    """AV matmuls + x0 = den*q + att for one i-tile."""
                    s, ic, pt = ep["s"], ep["ic"], ep["pt"]
                    attt = ps_att.tile([128, 512], f32, tag="att")
                    att = attt[:, 0:VW]
                    for jp in range(NJT // 2):
                        lhs = pt[:, jp * 2 * ICW:(jp + 1) * 2 * ICW] \
                            .rearrange("p (two i) -> p two i", two=2) \
                            [:, :, il * 128:(il + 1) * 128]
                        vb = (s * NJT + 2 * jp) * VW
                        rhs = v_sb[:, vb:vb + 2 * VW] \
                            .rearrange("p (two d) -> p two d", two=2)
                        nc.tensor.matmul(att, lhs, rhs,
                                         start=(jp == 0),
                                         stop=(jp == NJT // 2 - 1),
                                         perf_mode=DR)
                    row0 = s * LQ + (ic * NIL + il) * 128
                    if il % 4 == 0:
                        qt4 = q_pool.tile([128, 4 * HID], bf16, tag="q")
                        nc.sync.dma_start(
                            qt4[:].rearrange("p (f d) -> p f d", f=4),
                            qbf_a[row0:row0 + 512, :]
                            .rearrange("(f p) d -> p f d", f=4))
                        ep["qt4"] = qt4
                    qt = ep["qt4"][:, (il % 4) * HID:(il % 4 + 1) * HID]
                    # x0 = den*q + att  (LN is row-scale invariant)
                    x0 = ep["xs"][:, il * HID:(il + 1) * HID]
                    nc.vector.scalar_tensor_tensor(
                        x0, qt, att[:, HID:HID + 1].opt(),
                        att[:, 0:HID], op0=Alu.mult, op1=Alu.add)
                    nc.vector.bn_stats(ep["mv6"][:, 6 * il:6 * il + 6], x0)
                    nc.vector.bn_aggr(ep["mva0"][:, 2 * il:2 * il + 2],
                                      ep["mv6"][:, 6 * il:6 * il + 6])

                def emit_mid(ep):
                    """rstd ladder for the whole chunk (stats done in
                    emit_a so the ladder's deps are ready -> the ACT queue
                    never stalls ahead of the next exps)."""
                    mva0 = ep["mva0"]
                    ln8a = st8_pool.tile([128, NIL], f32, tag="ln8a")
                    nc.scalar.activation(
                        ln8a[:].rearrange("p (t o) -> p t o", o=1),
                        mva0[:].rearrange("p (t o) -> p t o", o=2)
                        [:, :, 1:2],
                        AF.Ln, bias=eps_sb[:])
                    rstd8a = st8_pool.tile([128, NIL], f32, tag="r8a")
                    nc.scalar.activation(rstd8a[:], ln8a[:], AF.Exp,
                                         scale=-0.5)
                    ep["rstd8a"] = rstd8a

                def emit_z(ep, il):
                    """z = (x0-m)*rstd on DVE; every 4 i-tiles one batched
                    xbar DMA transpose (consumed by emit_fc one pipeline
                    iteration later)."""
                    mva0, rstd8a = ep["mva0"], ep["rstd8a"]
                    x0 = ep["xs"][:, il * HID:(il + 1) * HID]
                    z = ep["zs"][:, il * HID:(il + 1) * HID]
                    nc.vector.tensor_scalar(
                        z, x0, mva0[:, 2 * il:2 * il + 1].opt(),
                        rstd8a[:, il:il + 1].opt(),
                        op0=Alu.subtract, op1=Alu.mult)
                    if apply0:
                        z2 = z_pool.tile([128, HID], bf16, tag="z2")
                        nc.gpsimd.tensor_tensor(z2, z, n0w_sb[:],
                                                op=Alu.mult)
                        nc.gpsimd.tensor_tensor(z, z2, n0b_sb[:],
                                                op=Alu.add)
                    bt = 2 if ep.get("lastc") else 4
                    if il % bt == bt - 1:
                        lo = (il - bt + 1) * HID
                        hi = (il + 1) * HID
                        nc.sync.dma_start_transpose(
                            ep["zTs"][:, lo:hi]
                            .rearrange("p (c t) -> p c t", t=128),
                            ep["zs"][:, lo:hi])

                def emit_fc(ep, il):
                    """fc matmuls from the pre-transposed zT + relu-residual
                    + LN1 input."""
                    zf = ep["zs"][:, il * HID:(il + 1) * HID]
                    zT = ep["zTs"][:, il * HID:(il + 1) * HID]
                    hrest = ps_fc.tile([128, 512], f32, tag="fc")
                    hres = hrest[:, 0:HID]
                    nc.tensor.matmul(hres, one_sb[:], fb_sb[:],
                                     start=True, stop=False)
                    for hh in range(2):
                        nc.tensor.matmul(
                            hres, zT[:, hh * 128:(hh + 1) * 128],
                            fw_sb[:, hh * HID:(hh + 1) * HID],
                            start=False, stop=(hh == 1))
                    y0 = ep["ys"][:, il * HID:(il + 1) * HID]
                    nc.vector.scalar_tensor_tensor(
                        y0, hres, 0.0, zf,
                        op0=Alu.max, op1=Alu.add)
                    nc.vector.bn_stats(ep["mv6b"][:, 6 * il:6 * il + 6], y0)
                    nc.vector.bn_aggr(ep["mva1"][:, 2 * il:2 * il + 2],
                                      ep["mv6b"][:, 6 * il:6 * il + 6])

                def emit_end(ep, last=False):
                    """LN1 rstd ladder, final scale, store. On the last
                    chunk the normalize alternates DVE/ACT and the stores
                    alternate sync/scalar queues so the drain runs its
                    engines in parallel."""
                    s, ic = ep["s"], ep["ic"]
                    ys_t = ep["ys"]
                    mva1 = ep["mva1"]
                    ln8b = st8_pool.tile([128, NIL], f32, tag="ln8b")
                    nc.scalar.activation(
                        ln8b[:].rearrange("p (t o) -> p t o", o=1),
                        mva1[:].rearrange("p (t o) -> p t o", o=2)[:, :, 1:2],
                        AF.Ln, bias=eps_sb[:])
                    rstd8b = st8_pool.tile([128, NIL], f32, tag="r8b")
                    nc.scalar.activation(rstd8b[:], ln8b[:], AF.Exp,
                                         scale=-0.5)
                    for il in range(NIL):
                        if il % 2 == 0:
                            ot2 = o_pool.tile([128, 2 * HID], bf16,
                                              tag="ot")
                        dst = ot2[:, (il % 2) * HID:(il % 2 + 1) * HID]
                        ys_il = ys_t[:, il * HID:(il + 1) * HID]
                        if last and il % 2 == 1:
                            b1 = st8_pool.tile([128, 1], f32, tag="b1")
                            nc.vector.tensor_scalar(
                                b1[:], mva1[:, 2 * il:2 * il + 1],
                                rstd8b[:, il:il + 1].opt(), -1.0,
                                op0=Alu.mult, op1=Alu.mult)
                            nc.scalar.activation(
                                dst, ys_il, AF.Identity, bias=b1[:],
                                scale=rstd8b[:, il:il + 1].opt())
                        else:
                            nc.vector.tensor_scalar(
                                dst, ys_il,
                                mva1[:, 2 * il:2 * il + 1].opt(),
                                rstd8b[:, il:il + 1].opt(),
                                op0=Alu.subtract, op1=Alu.mult)
                        if il % 2 == 1:
                            row0e = s * LQ + (ic * NIL + il - 1) * 128
                            eng = nc.scalar if (last and (il // 2) % 2) \
                                else nc.sync
                            eng.dma_start(
                                out_a[row0e:row0e + 256, :]
                                .rearrange("(f p) d -> p f d", f=2),
                                ot2[:].rearrange("p (f d) -> p f d", f=2))

                chunks = [(s, ic) for s in range(SEGS) for ic in range(NIC)]
                # 4-stage pipeline: iteration ci emits, per i-tile k: scores+
                # exp of chunk ci, AV+x0 of ci-1, fc+y of ci-2 (consuming the
                # zT transposed at the END of iteration ci-1 -> a full
                # iteration of DMA slack); then LN0 stats+z+transpose for
                # ci-1 and the LN1 epilogue of ci-3.
                prev1 = prev2 = prev3 = None
                for ci in range(len(chunks) + 3):
                    cur = None
                    if ci < len(chunks):
                        s, ic = chunks[ci]
                        cur = {
                            "s": s, "ic": ic,
                            "lastc": ci == len(chunks) - 1,
                            "pt": pt_pool.tile([128, NJT * ICW], f8,
                                               tag="pt", name="pt"),
                            "xs": xs_pool.tile([128, NIL * HID], bf16,
                                               tag="xs", name="xs"),
                            "ys": ys_pool.tile([128, NIL * HID], bf16,
                                               tag="ys", name="ys"),
                            "zs": z_pool.tile([128, NIL * HID], bf16,
                                              tag="zs", name="zs"),
                            "zTs": zT_pool.tile([128, NIL * HID], bf16,
                                                tag="zTs", name="zTs"),
                            "mv6": st8_pool.tile([128, 6 * NIL], f32,
                                                 tag="mv6", name="mv6"),
                            "mva0": st8_pool.tile([128, 2 * NIL], f32,
                                                  tag="mva0", name="mva0"),
                            "mv6b": st8_pool.tile([128, 6 * NIL], f32,
                                                  tag="mv6b", name="mv6b"),
                            "mva1": st8_pool.tile([128, 2 * NIL], f32,
                                                  tag="mva1", name="mva1"),
                        }
                    for k in range(NIL):
                        if prev1 is not None:
                            emit_a(prev1, k)
                        if prev2 is not None:
                            emit_fc(prev2, k)
                        if cur is not None:
                            emit_score_pair(cur, k)
                        if ci == 0:
                            emit_proj_piece(1, 2 * k)
                            emit_proj_piece(1, 2 * k + 1)
                    if prev1 is not None:
                        emit_mid(prev1)
                        for k in range(NIL):
                            emit_z(prev1, k)
                    if prev3 is not None:
                        emit_end(prev3, last=(ci == len(chunks) + 2))
                    prev1, prev2, prev3 = cur, prev1, prev2

    nc.compile()
    return nc


def _get_nc(apply0: bool):
    key = (bool(apply0),)
    if key not in _built:
        _built[key] = _build(apply0)
    return _built[key]


def _shard(inputs, apply0):
    from concourse import mybir
    bf = mybir.dt.np(mybir.dt.bfloat16)
    f8 = mybir.dt.np(mybir.dt.float8e4)

    q = np.ascontiguousarray(np.asarray(inputs["q"], dtype=np.float32))
    h = np.ascontiguousarray(np.asarray(inputs["h"], dtype=np.float32))
    WQ = np.asarray(inputs["WQ"], dtype=np.float32)
    WK = np.asarray(inputs["WK"], dtype=np.float32)
    WV = np.asarray(inputs["WV"], dtype=np.float32)
    fcw = np.asarray(inputs["fc_w"], dtype=np.float32)
    fcb = np.asarray(inputs["fc_b"], dtype=np.float32)

    def to8(x):
        return np.clip(x, -240.0, 240.0).astype(f8)

    def pack8(wT, m):
        # wT [HID, m] -> [128, 2, m] -> [128, 2*m] fp8 (e-chunks adjacent)
        return np.ascontiguousarray(
            wT.reshape(2, 128, m).transpose(1, 0, 2).reshape(128, 2 * m)
        ).astype(f8)

    WQ8 = pack8(np.ascontiguousarray(WQ.T), QD)
    WK8 = pack8(np.ascontiguousarray(WK.T), QD)
    WV8 = pack8(np.ascontiguousarray(WV.T), HID)
    FCWT = np.ascontiguousarray(fcw.T).astype(bf)
    FCB = np.ascontiguousarray(fcb.reshape(1, HID)).astype(bf)

    in_maps = []
    for c in range(NCORES):
        sl = slice(c * ROWS, (c + 1) * ROWS)
        m = {
            "qT8": to8(np.ascontiguousarray(q[sl].T)),
            "hT8": to8(np.ascontiguousarray(h[sl].T)),
            "qbf": np.ascontiguousarray(q[sl]).astype(bf),
            "WQ8": WQ8, "WK8": WK8, "WV8": WV8,
            "FCWT": FCWT, "FCB": FCB,
        }
        if apply0:
            m["N0W"] = np.ascontiguousarray(
                np.broadcast_to(np.asarray(inputs["norm0_w"], np.float32),
                                (128, HID)))
            m["N0B"] = np.ascontiguousarray(
                np.broadcast_to(np.asarray(inputs["norm0_b"], np.float32),
                                (128, HID)))
        in_maps.append(m)
    return in_maps


def _run(inputs, trace=False, tmpdir=None):
    from concourse import bass_utils

    n0w = np.asarray(inputs["norm0_w"], np.float32)
    n0b = np.asarray(inputs["norm0_b"], np.float32)
    n1w = np.asarray(inputs["norm1_w"], np.float32)
    n1b = np.asarray(inputs["norm1_b"], np.float32)
    apply0 = not (np.allclose(n0w, 1.0) and np.allclose(n0b, 0.0))
    apply1 = not (np.allclose(n1w, 1.0) and np.allclose(n1b, 0.0))

    nc = _get_nc(apply0)
    in_maps = _shard(inputs, apply0)
    res = bass_utils.run_bass_kernel_spmd(
        nc, in_maps, core_ids=list(range(NCORES)), trace=trace,
        tmpdir=tmpdir)
    out = np.concatenate([np.asarray(res.results[c]["out"])
                          for c in range(NCORES)], axis=0).astype(np.float32)
    if apply1:
        out = out * n1w[None, :] + n1b[None, :]
    return out, res


def kernel(**inputs):
    out, _ = _run(inputs, trace=False)
    return out
